# revision 18
# baseline (speedup 1.0000x reference)
"""Trainium2 Bass kernel for nn_Comm_OUT (MTRNN -> Ted_Conv1d -> proj -> comm mask).

Data-parallel over N = E*S = 2048 sequences across 8 NeuronCores (256 each).
Per core, fully fused in SBUF:
  phase 0: xw = x @ Wx                      (transposed layout: H on partitions)
  phase 1+2 interleaved: 32-step MTRNN h = tanh(xw + h @ Wh + b) writing a
           12-slot ring of hidden states, with the 4 parallel convs
           (k=1,3,5,7, reflect padding) consuming the ring as shifted matmuls
           paced by ring WAR dependencies; PReLU fused into PSUM eviction
  phase 3: projection to C=64 logits per position
  phase 4: comm mask = "no end token (argmax==0) strictly before l",
           computed with free-axis reductions + running-max scan
All matmul operands are bf16 (fp32 PSUM accumulate): same PE rate as fp32r
but FWL halves LDWEIGHTS (hides under the 256-free RNN streams) and weight
DMA bytes halve.  DMAs are ordered xt+wx -> wh -> conv weights so phase 0
starts on the first arriving chunk.
"""

import numpy as np
import ml_dtypes

import concourse.bass as bass
import concourse.mybir as mybir
from concourse.tile import TileContext
from concourse.bass_utils import run_bass_kernel_spmd

F32 = mybir.dt.float32
BF16 = mybir.dt.bfloat16
AF = mybir.ActivationFunctionType
ALU = mybir.AluOpType

E, S, L, H, D_IN, C = 32, 64, 32, 512, 1536, 64
N = E * S
NCORES = 8
NC_N = N // NCORES          # 256 rows per core
HC = H // 128               # 4 H chunks
DC = D_IN // 128            # 12 D_IN chunks
TL = 2                      # output-l positions per conv PSUM tile
RING = L                    # full hidden-state history in SBUF (no wraps)
KS = [1, 3, 5, 7]

_uid = [0]


def _split_excess_waits(nc, limit=1):
    """walrus in this toolchain accepts at most one sem-wait per instruction;
    move excess waits onto same-engine no-ops inserted just before."""
    for f in nc.m.functions:
        for bb in f.blocks:
            insts = bb.instructions
            i = 0
            while i < len(insts):
                inst = insts[i]
                si = inst.sync_info
                waits = list(si.on_wait) if si and si.on_wait else []
                if len(waits) > limit:
                    excess, keep = waits[:-limit], waits[-limit:]
                    inst.sync_info = mybir.SyncInfo(
                        on_wait=keep, on_update=list(si.on_update or []))
                    pos = i
                    for j in range(0, len(excess), limit):
                        _uid[0] += 1
                        nop = mybir.InstNoOp(
                            name=f"I-waitsplit-{_uid[0]}", ins=[], outs=[])
                        nop.engine = inst.engine
                        nop.bass_nofuse = True
                        nop.sync_info = mybir.SyncInfo(
                            on_wait=excess[j:j + limit], on_update=[])
                        insts.insert(pos, nop)
                        nc.register_instruction(nop, overwrite=True)
                        pos += 1
                        i += 1
                i += 1
            bb.instructions = insts


def _reflect(i):
    if i < 0:
        return -i
    if i > L - 1:
        return 2 * (L - 1) - i
    return i


def _conv_mm_plan():
    """Per (ltile, conv): ordered list of (dk, kc, slot0, n_l, out_j) matmuls
    over the hidden-state history. Runs split only on non-contiguity
    (reflection). The identity tap (offset 0) goes first, so the first
    matmul of every PSUM accumulation group covers the full tile."""
    plans = {}
    for ci, k in enumerate(KS):
        p = (k - 1) // 2
        taps = sorted(range(k), key=lambda dk: (dk - p != 0, dk))
        for lt in range(L // TL):
            l0 = TL * lt
            mms = []
            for dk in taps:
                o = dk - p
                ins = [_reflect(l0 + j + o) for j in range(TL)]
                # reflection only ever yields +1- or -1-consecutive pairs
                # at TL=2; a -1 pair becomes one reversed-stride matmul
                if TL == 2 and ins[1] == ins[0] - 1:
                    runs = [(0, TL, True)]
                else:
                    runs = []
                    j = 0
                    while j < TL:
                        j2 = j
                        while j2 + 1 < TL and ins[j2 + 1] == ins[j2] + 1:
                            j2 += 1
                        runs.append((j, j2 - j + 1, False))
                        j = j2 + 1
                for kc in range(HC):
                    for (j, nl, rev) in runs:
                        mms.append((dk, kc, ins[j], nl, j, rev))
            plans[(lt, ci)] = mms
    return plans


def build_nc(prelu_a: float, rep: int = 1):
    nc = bass.Bass()

    xt_d = nc.declare_dram_parameter("xt", [128, DC, NC_N], BF16, isOutput=False)
    wx_d = nc.declare_dram_parameter("wx", [128, DC, HC, 128], BF16, isOutput=False)
    wh_d = nc.declare_dram_parameter("wh", [128, HC, HC, 128], BF16, isOutput=False)
    wc_d = [nc.declare_dram_parameter(f"wc{k}", [128, k, HC, 128], BF16,
                                      isOutput=False) for k in KS]
    wo_d = nc.declare_dram_parameter("wo", [128, HC, C], BF16, isOutput=False)
    bsum_d = nc.declare_dram_parameter("bsum", [128, HC], F32, isOutput=False)
    cb_d = nc.declare_dram_parameter("cb", [128, HC], F32, isOutput=False)
    bout_d = nc.declare_dram_parameter("bout", [128, C], F32, isOutput=False)
    out_d = nc.declare_dram_parameter("out", [NC_N, L, C], BF16, isOutput=True)

    plans = _conv_mm_plan()

    with TileContext(nc) as tc:
        with (
            tc.tile_pool(name="const", bufs=1) as cpool,
            tc.tile_pool(name="main", bufs=1) as mpool,
            tc.tile_pool(name="yt", bufs=2) as ypool,
            tc.tile_pool(name="msk", bufs=1) as kpool,
            tc.tile_pool(name="ph0", bufs=1) as p0pool,
        ):
            # ---- input/weight loads, ordered by first use; few large
            # DMAs (each dma_start trigger costs ~650ns on the sync queue,
            # so trigger count — not bandwidth — gates the startup) ----
            DCH = 4                              # d-chunks per phase-0 DMA
            wx_sb = p0pool.tile([128, DC, HC, 128], BF16, tag="wx", name="wx")
            xt_sb = p0pool.tile([128, DC, NC_N], BF16, tag="xt", name="xt")
            wh_sb = cpool.tile([128, HC, HC, 128], BF16, tag="wh", name="wh")
            bsum_sb = cpool.tile([128, HC], F32, tag="bsum", name="bsum")
            # first phase-0 chunk, then RNN weights, then the rest
            nc.sync.dma_start(out=xt_sb[:, 0:DCH, :], in_=xt_d[:, 0:DCH, :])
            nc.sync.dma_start(out=wx_sb[:, 0:DCH, :, :], in_=wx_d[:, 0:DCH, :, :])
            nc.sync.dma_start(out=wh_sb[:], in_=wh_d[:, :, :, :])
            nc.sync.dma_start(out=bsum_sb[:], in_=bsum_d[:, :])
            for d0 in range(DCH, DC, DCH):
                nc.sync.dma_start(out=xt_sb[:, d0:d0 + DCH, :],
                                  in_=xt_d[:, d0:d0 + DCH, :])
                nc.sync.dma_start(out=wx_sb[:, d0:d0 + DCH, :, :],
                                  in_=wx_d[:, d0:d0 + DCH, :, :])
            wc_sb = []
            for i, k in enumerate(KS):
                t = cpool.tile([128, k, HC, 128], BF16, tag=f"wc{k}", name=f"wc{k}")
                nc.sync.dma_start(out=t[:], in_=wc_d[i][:, :, :, :])
                wc_sb.append(t)
            cb_sb = cpool.tile([128, HC], F32, tag="cb", name="cb")
            nc.sync.dma_start(out=cb_sb[:], in_=cb_d[:, :])
            wo_sb = cpool.tile([128, HC, C], BF16, tag="wo", name="wo")
            nc.sync.dma_start(out=wo_sb[:], in_=wo_d[:, :, :])
            bout_bc = cpool.tile([128, C], F32, tag="boutbc", name="boutbc")
            nc.sync.dma_start(out=bout_bc[:], in_=bout_d[:, :])

            # ---- persistent state ----
            hs = [mpool.tile([128, RING, NC_N], BF16, tag=f"hs{m}", name=f"hs{m}")
                  for m in range(HC)]
            xw = [mpool.tile([128, NC_N], F32, tag=f"xw{m}", name=f"xw{m}")
                  for m in range(HC)]
            P = [mpool.tile([128, L, C], BF16, tag=f"P{h}", name=f"P{h}")
                 for h in range(2)]
            emax = [kpool.tile([128, L], F32, tag=f"emax{h}", name=f"emax{h}")
                    for h in range(2)]
            notend = [kpool.tile([128, L], F32, tag=f"ne{h}", name=f"ne{h}")
                      for h in range(2)]
            run = [kpool.tile([128, 1], F32, tag=f"run{h}", name=f"run{h}")
                   for h in range(2)]

            def rnn_step(t, ps1):
                for m in range(HC):
                    if t == 0:
                        nc.scalar.activation(hs[m][:, 0, :], xw[m][:], AF.Tanh,
                                             bias=bsum_sb[:, m:m + 1])
                        continue
                    ps = ps1.tile([128, NC_N], F32, tag="ps1", name="ps1")
                    for kc in range(HC):
                        nc.tensor.matmul(ps[:], wh_sb[:, kc, m, :],
                                         hs[kc][:, t - 1, :],
                                         start=(kc == 0), stop=(kc == HC - 1))
                    tmp = ypool.tile([128, NC_N], F32, tag="rnntmp",
                                     name="rnntmp", bufs=3)
                    nc.vector.tensor_tensor(tmp[:], ps[:], xw[m][:], op=ALU.add)
                    nc.scalar.activation(hs[m][:, t, :], tmp[:], AF.Tanh,
                                         bias=bsum_sb[:, m:m + 1])

            def conv_groups(lt, cis, ps2, yts):
                for ci in cis:
                    psc = ps2.tile([128, TL, NC_N], F32, tag="psc", name="psc")
                    mms = plans[(lt, ci)]
                    nmm = len(mms)
                    for idx, (dk, kc, s0, n_l, out_j, rev) in enumerate(mms):
                        dst = psc[:, :, :] if n_l == TL else \
                            psc[:, out_j:out_j + 1, :]
                        if rev:
                            src = hs[kc][:, s0::-1, :] if s0 == n_l - 1 \
                                else hs[kc][:, s0:s0 - n_l:-1, :]
                        else:
                            src = hs[kc][:, s0:s0 + n_l, :]
                        nc.tensor.matmul(
                            dst, wc_sb[ci][:, dk, kc, :], src,
                            start=(idx == 0), stop=(idx == nmm - 1))
                    yt = ypool.tile([128, TL, NC_N], BF16, tag=f"yt{ci}",
                                    name=f"yt{ci}")
                    nc.scalar.activation(yt[:], psc[:], AF.Prelu,
                                         bias=cb_sb[:, ci:ci + 1],
                                         alpha=float(prelu_a))
                    yts.append(yt)

            def proj_ltile(lt, ps3, yts):
                l0 = TL * lt
                for j in range(TL):
                    l = l0 + j
                    for h in range(2):
                        psp = ps3.tile([128, C], F32,
                                       tag="psp", name="psp")
                        for kc in range(HC):
                            nc.tensor.matmul(
                                psp[:],
                                yts[kc][:, j, h * 128:(h + 1) * 128],
                                wo_sb[:, kc, :],
                                start=(kc == 0), stop=(kc == HC - 1))
                        nc.vector.tensor_tensor(P[h][:, l, :], psp[:],
                                                bout_bc[:], op=ALU.add)
                        nc.vector.tensor_reduce(
                            emax[h][:, l:l + 1], P[h][:, l:l + 1, 1:],
                            axis=mybir.AxisListType.X, op=ALU.max)
                        # notend = (max_{c>=1} pred_c > pred_0); keep-mask
                        # run[l] = prod_{j<l} notend[j], multiplicative scan
                        nc.vector.tensor_tensor(
                            notend[h][:, l:l + 1], emax[h][:, l:l + 1],
                            P[h][:, l, 0:1], op=ALU.is_gt)
                        nc.vector.tensor_scalar(
                            P[h][:, l, :], P[h][:, l, :],
                            run[h][:], None, ALU.mult)
                        nc.vector.tensor_tensor(
                            run[h][:], run[h][:], notend[h][:, l:l + 1],
                            op=ALU.mult)
                for h in range(2):
                    nc.sync.dma_start(
                        out=out_d[h * 128:(h + 1) * 128, l0:l0 + TL, :],
                        in_=P[h][:, l0:l0 + TL, :])

            for _ in range(rep):
                # ---- phase 0: xw = x @ Wx (d-outer so MMs start on the
                # first arriving DMA chunk; 4 PSUM tiles live) ----
                with tc.tile_pool(name="ps0", bufs=1, space="PSUM") as ps0:
                    ps_x = [ps0.tile([128, NC_N], F32, tag=f"ps0{m}",
                                     name=f"ps0{m}") for m in range(HC)]
                    for d in range(DC):
                        for m in range(HC):
                            nc.tensor.matmul(ps_x[m][:], wx_sb[:, d, m, :],
                                             xt_sb[:, d, :],
                                             start=(d == 0), stop=(d == DC - 1))
                    for m in range(HC):
                        nc.vector.tensor_copy(xw[m][:], ps_x[m][:])

                # ---- interleaved RNN + conv + proj ----
                with (
                    tc.tile_pool(name="ps1", bufs=3, space="PSUM") as ps1,
                    tc.tile_pool(name="ps2", bufs=3, space="PSUM") as ps2,
                    tc.tile_pool(name="ps3", bufs=2, space="PSUM") as ps3,
                ):
                    for h in range(2):
                        nc.vector.memset(run[h][:], 1.0)
                    # pre-run 3 steps, then 2 steps per l-tile placed so
                    # each conv group's newest-h tap is ready just in time:
                    # ci0/ci1 need h<=2lt+2, ci2 needs h_{2lt+3} (rnn_a),
                    # ci3 needs h_{2lt+4} (rnn_b)
                    PRE = 3
                    for t in range(PRE):
                        rnn_step(t, ps1)
                    nt = PRE
                    for lt in range(L // TL):
                        yts = []
                        conv_groups(lt, (0, 1), ps2, yts)
                        if nt < L:
                            rnn_step(nt, ps1)
                            nt += 1
                        conv_groups(lt, (2,), ps2, yts)
                        if nt < L:
                            rnn_step(nt, ps1)
                            nt += 1
                        conv_groups(lt, (3,), ps2, yts)
                        proj_ltile(lt, ps3, yts)

    _split_excess_waits(nc, limit=1)
    return nc


def _pack_inputs(inputs):
    """Host-side packing into PE-ready layouts (per-core + replicated).
    Matmul operands are packed as bf16; biases stay fp32."""
    bf = ml_dtypes.bfloat16
    x = inputs["h_w_action"].reshape(N, D_IN)
    wx = np.ascontiguousarray(
        inputs["Wx"].reshape(DC, 128, HC, 128).transpose(1, 0, 2, 3)).astype(bf)
    wh = np.ascontiguousarray(
        inputs["Wh"].reshape(HC, 128, HC, 128).transpose(1, 0, 2, 3)).astype(bf)
    wcs = {}
    for k in KS:
        w = inputs[f"conv_w{k}"]                      # (128, 512, k)
        wt = w.transpose(1, 2, 0).reshape(HC, 128, k, 128)
        wcs[k] = np.ascontiguousarray(wt.transpose(1, 2, 0, 3)).astype(bf)
    wo = np.ascontiguousarray(
        inputs["Wout"].reshape(HC, 128, C).transpose(1, 0, 2)).astype(bf)
    bsum = np.ascontiguousarray(
        (inputs["bx"] + inputs["bh"]).reshape(HC, 128).T)
    cb = np.ascontiguousarray(np.concatenate(
        [inputs[f"conv_b{k}"] for k in KS]).reshape(HC, 128).T)
    bout = np.ascontiguousarray(np.broadcast_to(inputs["bout"].reshape(1, C), (128, C)))

    in_maps = []
    for c in range(NCORES):
        xs = x[c * NC_N:(c + 1) * NC_N]               # (256, 1536)
        xt = np.ascontiguousarray(
            xs.T.reshape(DC, 128, NC_N).transpose(1, 0, 2)).astype(bf)
        m = {"xt": xt, "wx": wx, "wh": wh, "wo": wo,
             "bsum": bsum, "cb": cb, "bout": bout}
        for k in KS:
            m[f"wc{k}"] = wcs[k]
        in_maps.append(m)
    return in_maps


_NC_CACHE = {}
_RUNNER_CACHE = {}


def _make_runner(nc):
    """Persistent jitted PJRT runner (mirrors bass2jax.run_bass_via_pjrt's
    multi-core path) so repeat kernel() calls skip re-tracing."""
    import jax
    from jax.sharding import Mesh, PartitionSpec
    try:
        from jax.experimental.shard_map import shard_map
    except ImportError:
        from jax import shard_map
    from concourse import bass2jax

    bass2jax.install_neuronx_cc_hook()
    partition_name = (nc.partition_id_tensor.name
                      if nc.partition_id_tensor else None)
    in_names, out_names, out_avals, zero_outs = [], [], [], []
    for alloc in nc.m.functions[0].allocations:
        if not isinstance(alloc, mybir.MemoryLocationSet):
            continue
        name = alloc.memorylocations[0].name
        if alloc.kind == "ExternalInput":
            if name != partition_name:
                in_names.append(name)
        elif alloc.kind == "ExternalOutput":
            shape = tuple(alloc.tensor_shape)
            dtype = mybir.dt.np(alloc.dtype)
            out_names.append(name)
            out_avals.append(jax.core.ShapedArray(shape, dtype))
            zero_outs.append(np.zeros(shape, dtype))
    n_params, n_outs = len(in_names), len(out_avals)
    all_in_names = list(in_names) + list(out_names)
    if partition_name is not None:
        all_in_names.append(partition_name)

    def _body(*args):
        operands = list(args)
        if partition_name is not None:
            operands.append(bass2jax.partition_id_tensor())
        return tuple(bass2jax._bass_exec_p.bind(
            *operands,
            out_avals=tuple(out_avals),
            in_names=tuple(all_in_names),
            out_names=tuple(out_names),
            lowering_input_output_aliases=(),
            sim_require_finite=True,
            sim_require_nnan=True,
            nc=nc,
        ))

    devices = jax.devices()[:NCORES]
    mesh = Mesh(np.asarray(devices), ("core",))
    in_specs = (PartitionSpec("core"),) * (n_params + n_outs)
    out_specs = (PartitionSpec("core"),) * n_outs
    donate = tuple(range(n_params, n_params + n_outs))
    sharded = jax.jit(
        shard_map(_body, mesh=mesh, in_specs=in_specs, out_specs=out_specs,
                  check_rep=False),
        donate_argnums=donate, keep_unused=True)

    def call(in_maps):
        concat_in = [np.concatenate([np.asarray(in_maps[c][nm])
                                     for c in range(NCORES)], axis=0)
                     for nm in in_names]
        zeros = [np.zeros((NCORES * z.shape[0], *z.shape[1:]), z.dtype)
                 for z in zero_outs]
        out_arrs = sharded(*concat_in, *zeros)
        oidx = out_names.index("out")
        full = np.asarray(out_arrs[oidx])
        return full.reshape(NCORES, NC_N, L, C)

    return call


def kernel(**inputs) -> np.ndarray:
    inputs = {k: np.asarray(v, dtype=np.float32) for k, v in inputs.items()}
    prelu_a = float(np.asarray(inputs["prelu_a"]))
    key = (prelu_a, 1)
    if key not in _NC_CACHE:
        _NC_CACHE[key] = build_nc(prelu_a, rep=1)
    nc = _NC_CACHE[key]
    in_maps = _pack_inputs(inputs)
    try:
        if key not in _RUNNER_CACHE:
            _RUNNER_CACHE[key] = _make_runner(nc)
        out = _RUNNER_CACHE[key](in_maps)
    except Exception:
        res = run_bass_kernel_spmd(nc, in_maps, core_ids=list(range(NCORES)))
        out = np.stack([res.results[c]["out"] for c in range(NCORES)], axis=0)
    return out.reshape(E, S, L, C).astype(np.float32)
